# revision 34
# baseline (speedup 1.0000x reference)
"""Trainium2 Bass kernel for nn_Captioner_41412074668572 (retrieval_knn).

Computes: mean over (b, n) of min over l of ||image_features[b,n] - emb_table[token_ids[b,l]]||_2

Strategy (8 NeuronCores, data-parallel over batch B=32 -> 4 batches/core):
  y-stationary matmul: weights = (-2*y)^T tiles [d=128, l=128] (reused across
  2048 streamed x columns -> LDWEIGHTS amortized 16x vs the x-stationary
  form), rhs = x^T fp8 [d=128, n=512] slices, PSUM accumulates -2*x.y over
  the d=1024 contraction into [l=128, n=512] chunks.  With dr=True both
  operands are fp8 and pairs of k-chunks run as one DoubleRow matmul
  (2 MACs/cell/cycle).
  Epilogue: ACT copies PSUM->SBUF in bf16 adding the per-partition bias
  y2[l] - 2048 (centering d2' near 0 keeps bf16 quantization harmless);
  PE transposes each [128,128] block (data as stationary, identity moving);
  DVE min-reduces the transposed [n=128, 4, l=128] banks over l; then
  m2 = mins + (x2+2048), clamp, sqrt (+1 Newton step), row-sum -> [128,1].
  Host sums 8*[128] partials in float64 and divides by B*N.
"""

import numpy as np
import ml_dtypes

B, N, L, D, V = 32, 2048, 128, 1024, 32000
N_CORES = 8
B_LOC = B // N_CORES          # 4 batches per core
P = 128                       # partitions
KC = D // P                   # 8 contraction chunks of 128
NT = N // P                   # 16 n-tiles of 128 per batch
T = B_LOC * NT                # 64 mins columns per core
NCH = 4                       # n-chunks of 512 per batch
CHW = N // NCH                # chunk width 512

_CACHE: dict = {}

BF16 = ml_dtypes.bfloat16
FP8 = ml_dtypes.float8_e4m3


DEFAULT_KNOBS = dict(
    dr=True,          # fp8 DoubleRow mains (both operands fp8)
    xmode="two",      # "two": 2x4.2MB batch-pair DMAs on sync; "one": 8.4MB;
                      # "sync1": per-batch on sync; "whole": dual-queue
    q3=False,         # spread x over all 3 DMA queues (else 2-queue alternate)
    bufs_x0=4,        # x slots, sync-queue pool (pairs in "two" mode)
    bufs_x1=4,        # whole-batch x slots, scalar-queue pool (non-sync1)
    bufs_x2=2,        # whole-batch x slots, gpsimd-queue pool (q3 only)
    bufs_xh=2,        # half-batch x slots (ksplit / q3-b2 pools)
    bufs_d2=3,
    bufs_ps=4,        # main psum chunk tiles
    bufs_pt=3,        # transpose psum tiles
    y_eng="gpsimd",   # engine for the y DMA (off the HWDGE x queues)
    out_last=True,    # emit the (512B) out DMA only on the last rep
)


def _build_nc(reps: int = 1, **knobs):
    """Build the Bass program. `reps` unrolls the whole body N times inside
    one NEFF (used only for marginal-time measurement in test.py)."""
    import concourse.tile as tile
    from concourse import bacc, mybir
    from concourse.masks import make_identity

    kn = dict(DEFAULT_KNOBS)
    kn.update(knobs)

    f32 = mybir.dt.float32
    bf16 = mybir.dt.bfloat16
    fp8 = mybir.dt.float8e4

    dr = kn["dr"]

    nc = bacc.Bacc("TRN2", target_bir_lowering=False, debug=False,
                   num_devices=N_CORES)

    # x^T fp8: [b, p, k, n] with x[b, n, k*128+p]
    if kn["xmode"] in ("one", "two"):
        # partition-outermost over ALL batches: one/two giant contiguous
        # DMAs (per-transfer fixed overhead ~0.67us caps 2MB transfers at
        # ~370 GB/s; 4-8MB transfers reach ~400-425)
        xt1 = nc.dram_tensor("xt1", [P, B_LOC, KC, N], fp8,
                             kind="ExternalInput")
    else:
        xt = nc.dram_tensor("xt", [B_LOC, P, KC, N], fp8,
                            kind="ExternalInput")
    if dr:
        # y packed for DoubleRow: [p, b, k2, ko, l] = -2*y[b, l, (2*k2+ko)*128+p]
        yt = nc.dram_tensor("ytdr", [P, B_LOC, KC // 2, 2, L], fp8,
                            kind="ExternalInput")
    else:
        # y bf16: [p, b, k, l] = -2*y[b, l, k*128+p]
        yt = nc.dram_tensor("ytb", [P, B_LOC, KC, L], bf16,
                            kind="ExternalInput")
    # aux: cols [0, T) = x2 + 2048 arranged [p, b*NT + t] matching mins
    # columns; cols [T, T + B_LOC) = y2[l] - 2048 per batch (partition = l).
    aux = nc.dram_tensor("aux", [P, T + B_LOC], f32, kind="ExternalInput")
    out = nc.dram_tensor("out", [P, 1], f32, kind="ExternalOutput")

    with tile.TileContext(nc) as tc:
        with (
            tc.tile_pool(name="xp0", bufs=kn["bufs_x0"]) as xp0,
            tc.tile_pool(name="xp1", bufs=kn["bufs_x1"]) as xp1,
            tc.tile_pool(name="xp2", bufs=kn["bufs_x2"]) as xp2,
            tc.tile_pool(name="xph0", bufs=kn["bufs_xh"]) as xph0,
            tc.tile_pool(name="xph1", bufs=kn["bufs_xh"]) as xph1,
            tc.tile_pool(name="yp", bufs=2) as yp,
            tc.tile_pool(name="cons", bufs=2) as cons,
            tc.tile_pool(name="idn", bufs=1) as idn,
            tc.tile_pool(name="d2p", bufs=kn["bufs_d2"]) as d2p,
            tc.tile_pool(name="ps", bufs=kn["bufs_ps"], space="PSUM") as pp,
            tc.tile_pool(name="pt", bufs=kn["bufs_pt"], space="PSUM") as pt,
        ):
            ident = idn.tile([P, P], bf16, tag="ident")
            make_identity(nc, ident[:])

            def emit_body(last: bool):
                # small constants first (off the HWDGE rings)
                auxs = cons.tile([P, T + B_LOC], f32, tag="auxs")
                nc.gpsimd.dma_start(auxs[:], aux[:])
                x2s = auxs[:, 0:T]
                y2s = auxs[:, T:T + B_LOC]
                if dr:
                    ytile = yp.tile([P, B_LOC, KC // 2, 2, L], fp8, tag="yt")
                else:
                    ytile = yp.tile([P, B_LOC, KC, L], bf16, tag="yt")
                getattr(nc, kn["y_eng"]).dma_start(ytile[:], yt[:])

                mins = cons.tile([P, T], f32, tag="mins")

                # x DMA plan: per batch a k-slice accessor rhs(k, nslice).
                # q3 spreads the 8.4 MB x stream over all three DMA queues
                # (sync / scalar / gpsimd-SWDGE): b0 sync, b1 scalar,
                # b2 k-split across sync+scalar, b3 gpsimd.
                xacc = []
                if kn["xmode"] == "one":
                    # one 8.4MB x DMA per rep (max transfer efficiency);
                    # bufs_x0=2 -> next rep's DMA streams during this rep
                    ta = xp0.tile([P, B_LOC, KC, N], fp8, tag="xt",
                                  name="xall")
                    nc.sync.dma_start(ta[:], xt1[:])
                    for b in range(B_LOC):
                        if dr:
                            xacc.append((lambda t, bb: lambda k2, ns:
                                         t[:, bb, 2 * k2:2 * k2 + 2, ns])(ta, b))
                        else:
                            xacc.append((lambda t, bb: lambda k, ns:
                                         t[:, bb, k, ns])(ta, b))
                elif kn["xmode"] == "two":
                    # two 4.2MB x DMAs per rep (batch pairs)
                    for g in range(2):
                        tg = xp0.tile([P, 2, KC, N], fp8, tag="xt",
                                      name="xpair")
                        nc.sync.dma_start(tg[:], xt1[:, 2 * g:2 * g + 2])
                        for bb in range(2):
                            if dr:
                                xacc.append((lambda t, b2: lambda k2, ns:
                                             t[:, b2, 2 * k2:2 * k2 + 2, ns])(tg, bb))
                            else:
                                xacc.append((lambda t, b2: lambda k, ns:
                                             t[:, b2, k, ns])(tg, bb))
                elif kn["xmode"] == "sync1":
                    # all x on the sync HWDGE ring: a single queue saturates
                    # all 16 SDMA engines, and the scalar engine then carries
                    # ONLY the ACTIVATEs -- a scalar-issued DMA that waits on
                    # a buffer-free semaphore would head-of-line-block the
                    # PSUM-draining ACTs behind it (strict 8-deep FIFO).
                    for b in range(B_LOC):
                        xtile = xp0.tile([P, KC, N], fp8, tag="xt", name="xs")
                        nc.sync.dma_start(xtile[:], xt[b])
                        if dr:
                            xacc.append((lambda t: lambda k2, ns:
                                         t[:, 2 * k2:2 * k2 + 2, ns])(xtile))
                        else:
                            xacc.append((lambda t: lambda k, ns:
                                         t[:, k, ns])(xtile))
                elif kn["xmode"] == "ksplit2":
                    # per batch: two 1MB k-half DMAs on the batch's queue, so
                    # the k2<2 matmuls can start when the first half lands
                    for b in range(B_LOC):
                        xpool = (xph0, xph1)[b % 2]
                        eng = (nc.sync, nc.scalar)[b % 2]
                        ta = xpool.tile([P, KC // 2, N], fp8, tag="xt",
                                        name=f"xs{b % 2}a")
                        eng.dma_start(ta[:], xt[b][:, 0:KC // 2])
                        tb = xpool.tile([P, KC // 2, N], fp8, tag="xt",
                                        name=f"xs{b % 2}b")
                        eng.dma_start(tb[:], xt[b][:, KC // 2:KC])
                        if dr:
                            xacc.append((lambda a, c: lambda k2, ns: (
                                a[:, 2 * k2:2 * k2 + 2, ns] if k2 < KC // 4
                                else c[:, 2 * k2 - KC // 2:2 * k2 - KC // 2 + 2, ns]
                            ))(ta, tb))
                        else:
                            xacc.append((lambda a, c: lambda k, ns: (
                                a[:, k, ns] if k < KC // 2
                                else c[:, k - KC // 2, ns]
                            ))(ta, tb))
                elif kn["xmode"] == "ksplit":
                    # every batch: k-chunks 0..3 on sync, 4..7 on scalar
                    for b in range(B_LOC):
                        ta = xph0.tile([P, KC // 2, N], fp8, tag="xt",
                                       name=f"xka{b}")
                        nc.sync.dma_start(ta[:], xt[b][:, 0:KC // 2])
                        tb = xph1.tile([P, KC // 2, N], fp8, tag="xt",
                                       name=f"xkb{b}")
                        nc.scalar.dma_start(tb[:], xt[b][:, KC // 2:KC])
                        if dr:
                            xacc.append((lambda a, c: lambda k2, ns: (
                                a[:, 2 * k2:2 * k2 + 2, ns] if k2 < KC // 4
                                else c[:, 2 * k2 - KC // 2:2 * k2 - KC // 2 + 2, ns]
                            ))(ta, tb))
                        else:
                            xacc.append((lambda a, c: lambda k, ns: (
                                a[:, k, ns] if k < KC // 2
                                else c[:, k - KC // 2, ns]
                            ))(ta, tb))
                elif kn["q3"]:
                    t0 = xp0.tile([P, KC, N], fp8, tag="xt", name="xb0")
                    nc.sync.dma_start(t0[:], xt[0])
                    t1 = xp1.tile([P, KC, N], fp8, tag="xt", name="xb1")
                    nc.scalar.dma_start(t1[:], xt[1])
                    t2a = xph0.tile([P, KC // 2, N], fp8, tag="xt", name="xb2a")
                    nc.sync.dma_start(t2a[:], xt[2][:, 0:KC // 2])
                    t2b = xph1.tile([P, KC // 2, N], fp8, tag="xt", name="xb2b")
                    nc.scalar.dma_start(t2b[:], xt[2][:, KC // 2:KC])
                    t3 = xp2.tile([P, KC, N], fp8, tag="xt", name="xb3")
                    nc.gpsimd.dma_start(t3[:], xt[3])

                    def acc_whole(t):
                        return lambda k, ns: t[:, k, ns]

                    def acc_whole2(t):
                        return lambda k2, ns: t[:, 2 * k2:2 * k2 + 2, ns]

                    def acc_split2(ta, tb):
                        return lambda k2, ns: (
                            ta[:, 2 * k2:2 * k2 + 2, ns] if k2 < KC // 4
                            else tb[:, 2 * k2 - KC // 2:2 * k2 - KC // 2 + 2, ns]
                        )

                    def acc_split1(ta, tb):
                        return lambda k, ns: (
                            ta[:, k, ns] if k < KC // 2
                            else tb[:, k - KC // 2, ns]
                        )

                    if dr:
                        xacc = [acc_whole2(t0), acc_whole2(t1),
                                acc_split2(t2a, t2b), acc_whole2(t3)]
                    else:
                        xacc = [acc_whole(t0), acc_whole(t1),
                                acc_split1(t2a, t2b), acc_whole(t3)]
                else:
                    for b in range(B_LOC):
                        xpool = (xp0, xp1)[b % 2]
                        xtile = xpool.tile([P, KC, N], fp8, tag="xt",
                                           name=f"xt{b % 2}")
                        eng = nc.scalar if b % 2 else nc.sync
                        eng.dma_start(xtile[:], xt[b])
                        if dr:
                            xacc.append(
                                (lambda t: lambda k2, ns:
                                 t[:, 2 * k2:2 * k2 + 2, ns])(xtile))
                        else:
                            xacc.append(
                                (lambda t: lambda k, ns: t[:, k, ns])(xtile))

                for b in range(B_LOC):
                    rhs = xacc[b]
                    # mains: k-outer so each weight-load serves NCH chunks
                    pss = [
                        pp.tile([P, CHW], f32, tag="ps", name=f"ps{i}")
                        for i in range(NCH)
                    ]
                    if dr:
                        for k2 in range(KC // 2):
                            for ci in range(NCH):
                                nc.tensor.matmul(
                                    pss[ci][:],
                                    ytile[:, b, k2, :, :],
                                    rhs(k2, slice(ci * CHW, (ci + 1) * CHW)),
                                    start=(k2 == 0),
                                    stop=(k2 == KC // 2 - 1),
                                    perf_mode=mybir.MatmulPerfMode.DoubleRow,
                                )
                    else:
                        for k in range(KC):
                            for ci in range(NCH):
                                nc.tensor.matmul(
                                    pss[ci][:],
                                    ytile[:, b, k, :],
                                    rhs(k, slice(ci * CHW, (ci + 1) * CHW)),
                                    start=(k == 0),
                                    stop=(k == KC - 1),
                                )
                    # epilogue per chunk
                    for ci in range(NCH):
                        d2 = d2p.tile([P, CHW], bf16, tag="d2")
                        nc.scalar.activation(
                            d2[:], pss[ci][:],
                            mybir.ActivationFunctionType.Identity,
                            bias=auxs[:, T + b:T + b + 1], scale=1.0,
                        )
                        pst = pt.tile([P, CHW // P, P], bf16, tag="pt")
                        for j in range(CHW // P):
                            nc.tensor.transpose(
                                pst[:, j, :],
                                d2[:, j * P:(j + 1) * P],
                                ident[:],
                            )
                        col = b * NT + ci * (CHW // P)
                        nc.vector.tensor_reduce(
                            mins[:, col:col + CHW // P], pst[:],
                            axis=mybir.AxisListType.X,
                            op=mybir.AluOpType.min,
                        )

                # post: d2min = mins + (x2+2048); cost = sqrt(max(d2min, eps));
                # one Newton step; row-sum -> [128, 1]
                m2 = cons.tile([P, T], f32, tag="m2")
                nc.vector.tensor_add(m2[:], mins[:], x2s)
                nc.vector.tensor_scalar_max(m2[:], m2[:], 1e-20)
                s = cons.tile([P, T], f32, tag="s")
                nc.scalar.sqrt(s[:], m2[:])
                r = cons.tile([P, T], f32, tag="r")
                nc.vector.reciprocal(r[:], s[:])
                t2 = cons.tile([P, T], f32, tag="t2")
                nc.vector.tensor_mul(t2[:], m2[:], r[:])
                nc.vector.tensor_add(t2[:], t2[:], s[:])
                ov = cons.tile([P, 1], f32, tag="ov")
                nc.vector.reduce_sum(ov[:], t2[:], axis=mybir.AxisListType.X)
                nc.vector.tensor_scalar_mul(ov[:], ov[:], 0.5)
                if last or not kn["out_last"]:
                    nc.sync.dma_start(out[:], ov[:])

            for ri in range(reps):
                emit_body(ri == reps - 1)

    nc.compile()
    return nc


def _get_nc(reps: int = 1, **knobs):
    key = ("nc", reps, tuple(sorted(knobs.items())))
    if key not in _CACHE:
        _CACHE[key] = _build_nc(reps, **knobs)
    return _CACHE[key]


def make_in_maps(image_features: np.ndarray, token_ids: np.ndarray,
                 emb_table: np.ndarray, **knobs) -> list[dict]:
    """Shard + lay out the full inputs into per-core device input maps."""
    x = np.asarray(image_features, dtype=np.float32)
    tok = np.asarray(token_ids)
    emb = np.asarray(emb_table, dtype=np.float32)

    in_maps = []
    for c in range(N_CORES):
        xc = x[c * B_LOC:(c + 1) * B_LOC]                       # [4, N, D]
        # x^T: [b, p, k, n] = x[b, n, k*128 + p]
        xT = np.ascontiguousarray(xc.transpose(0, 2, 1))        # [4, D, N]
        xT = xT.reshape(B_LOC, KC, P, N)
        xt_dev = np.ascontiguousarray(
            xT.transpose(0, 2, 1, 3)).astype(FP8)               # [4, P, KC, N]
        xt1_dev = np.ascontiguousarray(
            xT.transpose(2, 0, 1, 3)).astype(FP8)               # [P, 4, KC, N]
        # exact fp32 row norms (+2048), laid out [p, b*NT+t]
        x2 = np.square(xc).sum(axis=-1, dtype=np.float64) + 2048.0  # [4, N]
        x2t_dev = (
            x2.reshape(B_LOC, NT, P).transpose(2, 0, 1).reshape(P, T)
        ).astype(np.float32)

        y = emb[tok[c * B_LOC:(c + 1) * B_LOC]]                 # [4, L, D]
        yT = (-2.0 * y).transpose(0, 2, 1)                      # [4, D, L]
        yT = yT.reshape(B_LOC, KC, P, L)
        # DoubleRow fp8: [p, b, k2, ko, l]
        ytdr_dev = np.ascontiguousarray(
            yT.reshape(B_LOC, KC // 2, 2, P, L).transpose(3, 0, 1, 2, 4)
        ).astype(FP8)
        # bf16: [p, b, k, l]
        ytb_dev = np.ascontiguousarray(
            yT.transpose(2, 0, 1, 3)).astype(BF16)
        y2 = np.square(y).sum(axis=-1, dtype=np.float64) - 2048.0  # [4, L]
        y2t_dev = (y2.transpose(1, 0)).astype(np.float32)       # [L, 4]
        aux_dev = np.ascontiguousarray(
            np.concatenate([x2t_dev, y2t_dev], axis=1))         # [P, T + B_LOC]

        in_maps.append({
            "xt": xt_dev,
            "xt1": xt1_dev,
            "ytdr": ytdr_dev,
            "ytb": ytb_dev,
            "aux": aux_dev,
        })
    return in_maps


def kernel(image_features: np.ndarray, token_ids: np.ndarray,
           emb_table: np.ndarray) -> np.ndarray:
    from concourse import mybir
    from concourse.bass_utils import run_bass_kernel_spmd

    nc = _get_nc()
    declared = {
        alloc.memorylocations[0].name
        for alloc in nc.m.functions[0].allocations
        if isinstance(alloc, mybir.MemoryLocationSet)
        and alloc.kind == "ExternalInput"
    }
    in_maps = [
        {k: v for k, v in m.items() if k in declared}
        for m in make_in_maps(image_features, token_ids, emb_table)
    ]
    res = run_bass_kernel_spmd(nc, in_maps, core_ids=list(range(N_CORES)))
    total = np.float64(0.0)
    for c in range(N_CORES):
        total += res.results[c]["out"].astype(np.float64).sum()
    return np.float32(total / (B * N))


# revision 35
# speedup vs baseline: 1.0387x; 1.0387x over previous
"""Trainium2 Bass kernel for nn_Captioner_41412074668572 (retrieval_knn).

Computes: mean over (b, n) of min over l of ||image_features[b,n] - emb_table[token_ids[b,l]]||_2

Strategy (8 NeuronCores, data-parallel over batch B=32 -> 4 batches/core):
  y-stationary matmul: weights = (-2*y)^T tiles [d=128, l=128] (reused across
  2048 streamed x columns -> LDWEIGHTS amortized 16x vs the x-stationary
  form), rhs = x^T fp8 [d=128, n=512] slices, PSUM accumulates -2*x.y over
  the d=1024 contraction into [l=128, n=512] chunks.  With dr=True both
  operands are fp8 and pairs of k-chunks run as one DoubleRow matmul
  (2 MACs/cell/cycle).
  Epilogue: ACT copies PSUM->SBUF in bf16 adding the per-partition bias
  y2[l] - 2048 (centering d2' near 0 keeps bf16 quantization harmless);
  PE transposes each [128,128] block (data as stationary, identity moving);
  DVE min-reduces the transposed [n=128, 4, l=128] banks over l; then
  m2 = mins + (x2+2048), clamp, sqrt (+1 Newton step), row-sum -> [128,1].
  Host sums 8*[128] partials in float64 and divides by B*N.
"""

import numpy as np
import ml_dtypes

B, N, L, D, V = 32, 2048, 128, 1024, 32000
N_CORES = 8
B_LOC = B // N_CORES          # 4 batches per core
P = 128                       # partitions
KC = D // P                   # 8 contraction chunks of 128
NT = N // P                   # 16 n-tiles of 128 per batch
T = B_LOC * NT                # 64 mins columns per core
NCH = 4                       # n-chunks of 512 per batch
CHW = N // NCH                # chunk width 512

_CACHE: dict = {}

BF16 = ml_dtypes.bfloat16
FP8 = ml_dtypes.float8_e4m3


DEFAULT_KNOBS = dict(
    dr=True,          # fp8 DoubleRow mains (both operands fp8)
    xmode="two",      # "two": 2x4.2MB batch-pair DMAs on sync; "one": 8.4MB;
                      # "sync1": per-batch on sync; "whole": dual-queue
    q3=False,         # spread x over all 3 DMA queues (else 2-queue alternate)
    bufs_x0=5,        # x pair slots, sync-queue pool (2.5 reps of x buffered)
    bufs_x1=4,        # whole-batch x slots, scalar-queue pool (non-sync1)
    bufs_x2=2,        # whole-batch x slots, gpsimd-queue pool (q3 only)
    bufs_xh=2,        # half-batch x slots (ksplit / q3-b2 pools)
    bufs_d2=3,
    bufs_ps=4,        # main psum chunk tiles
    bufs_pt=3,        # transpose psum tiles
    y_eng="gpsimd",   # engine for the y DMA (off the HWDGE x queues)
    out_last=True,    # emit the (512B) out DMA only on the last rep
)


def _build_nc(reps: int = 1, **knobs):
    """Build the Bass program. `reps` unrolls the whole body N times inside
    one NEFF (used only for marginal-time measurement in test.py)."""
    import concourse.tile as tile
    from concourse import bacc, mybir
    from concourse.masks import make_identity

    kn = dict(DEFAULT_KNOBS)
    kn.update(knobs)

    f32 = mybir.dt.float32
    bf16 = mybir.dt.bfloat16
    fp8 = mybir.dt.float8e4

    dr = kn["dr"]

    nc = bacc.Bacc("TRN2", target_bir_lowering=False, debug=False,
                   num_devices=N_CORES)

    # x^T fp8: [b, p, k, n] with x[b, n, k*128+p]
    if kn["xmode"] in ("one", "two"):
        # partition-outermost over ALL batches: one/two giant contiguous
        # DMAs (per-transfer fixed overhead ~0.67us caps 2MB transfers at
        # ~370 GB/s; 4-8MB transfers reach ~400-425)
        xt1 = nc.dram_tensor("xt1", [P, B_LOC, KC, N], fp8,
                             kind="ExternalInput")
    else:
        xt = nc.dram_tensor("xt", [B_LOC, P, KC, N], fp8,
                            kind="ExternalInput")
    if dr:
        # y packed for DoubleRow: [p, b, k2, ko, l] = -2*y[b, l, (2*k2+ko)*128+p]
        yt = nc.dram_tensor("ytdr", [P, B_LOC, KC // 2, 2, L], fp8,
                            kind="ExternalInput")
    else:
        # y bf16: [p, b, k, l] = -2*y[b, l, k*128+p]
        yt = nc.dram_tensor("ytb", [P, B_LOC, KC, L], bf16,
                            kind="ExternalInput")
    # aux: cols [0, T) = x2 + 2048 arranged [p, b*NT + t] matching mins
    # columns; cols [T, T + B_LOC) = y2[l] - 2048 per batch (partition = l).
    aux = nc.dram_tensor("aux", [P, T + B_LOC], f32, kind="ExternalInput")
    out = nc.dram_tensor("out", [P, 1], f32, kind="ExternalOutput")

    with tile.TileContext(nc) as tc:
        with (
            tc.tile_pool(name="xp0", bufs=kn["bufs_x0"]) as xp0,
            tc.tile_pool(name="xp1", bufs=kn["bufs_x1"]) as xp1,
            tc.tile_pool(name="xp2", bufs=kn["bufs_x2"]) as xp2,
            tc.tile_pool(name="xph0", bufs=kn["bufs_xh"]) as xph0,
            tc.tile_pool(name="xph1", bufs=kn["bufs_xh"]) as xph1,
            tc.tile_pool(name="yp", bufs=2) as yp,
            tc.tile_pool(name="cons", bufs=2) as cons,
            tc.tile_pool(name="idn", bufs=1) as idn,
            tc.tile_pool(name="d2p", bufs=kn["bufs_d2"]) as d2p,
            tc.tile_pool(name="ps", bufs=kn["bufs_ps"], space="PSUM") as pp,
            tc.tile_pool(name="pt", bufs=kn["bufs_pt"], space="PSUM") as pt,
        ):
            ident = idn.tile([P, P], bf16, tag="ident")
            make_identity(nc, ident[:])

            def emit_body(last: bool):
                # small constants first (off the HWDGE rings)
                auxs = cons.tile([P, T + B_LOC], f32, tag="auxs")
                nc.gpsimd.dma_start(auxs[:], aux[:])
                x2s = auxs[:, 0:T]
                y2s = auxs[:, T:T + B_LOC]
                if dr:
                    ytile = yp.tile([P, B_LOC, KC // 2, 2, L], fp8, tag="yt")
                else:
                    ytile = yp.tile([P, B_LOC, KC, L], bf16, tag="yt")
                getattr(nc, kn["y_eng"]).dma_start(ytile[:], yt[:])

                mins = cons.tile([P, T], f32, tag="mins")

                # x DMA plan: per batch a k-slice accessor rhs(k, nslice).
                # q3 spreads the 8.4 MB x stream over all three DMA queues
                # (sync / scalar / gpsimd-SWDGE): b0 sync, b1 scalar,
                # b2 k-split across sync+scalar, b3 gpsimd.
                xacc = []
                if kn["xmode"] == "one":
                    # one 8.4MB x DMA per rep (max transfer efficiency);
                    # bufs_x0=2 -> next rep's DMA streams during this rep
                    ta = xp0.tile([P, B_LOC, KC, N], fp8, tag="xt",
                                  name="xall")
                    nc.sync.dma_start(ta[:], xt1[:])
                    for b in range(B_LOC):
                        if dr:
                            xacc.append((lambda t, bb: lambda k2, ns:
                                         t[:, bb, 2 * k2:2 * k2 + 2, ns])(ta, b))
                        else:
                            xacc.append((lambda t, bb: lambda k, ns:
                                         t[:, bb, k, ns])(ta, b))
                elif kn["xmode"] == "two":
                    # two 4.2MB x DMAs per rep (batch pairs)
                    for g in range(2):
                        tg = xp0.tile([P, 2, KC, N], fp8, tag="xt",
                                      name="xpair")
                        nc.sync.dma_start(tg[:], xt1[:, 2 * g:2 * g + 2])
                        for bb in range(2):
                            if dr:
                                xacc.append((lambda t, b2: lambda k2, ns:
                                             t[:, b2, 2 * k2:2 * k2 + 2, ns])(tg, bb))
                            else:
                                xacc.append((lambda t, b2: lambda k, ns:
                                             t[:, b2, k, ns])(tg, bb))
                elif kn["xmode"] == "sync1":
                    # all x on the sync HWDGE ring: a single queue saturates
                    # all 16 SDMA engines, and the scalar engine then carries
                    # ONLY the ACTIVATEs -- a scalar-issued DMA that waits on
                    # a buffer-free semaphore would head-of-line-block the
                    # PSUM-draining ACTs behind it (strict 8-deep FIFO).
                    for b in range(B_LOC):
                        xtile = xp0.tile([P, KC, N], fp8, tag="xt", name="xs")
                        nc.sync.dma_start(xtile[:], xt[b])
                        if dr:
                            xacc.append((lambda t: lambda k2, ns:
                                         t[:, 2 * k2:2 * k2 + 2, ns])(xtile))
                        else:
                            xacc.append((lambda t: lambda k, ns:
                                         t[:, k, ns])(xtile))
                elif kn["xmode"] == "ksplit2":
                    # per batch: two 1MB k-half DMAs on the batch's queue, so
                    # the k2<2 matmuls can start when the first half lands
                    for b in range(B_LOC):
                        xpool = (xph0, xph1)[b % 2]
                        eng = (nc.sync, nc.scalar)[b % 2]
                        ta = xpool.tile([P, KC // 2, N], fp8, tag="xt",
                                        name=f"xs{b % 2}a")
                        eng.dma_start(ta[:], xt[b][:, 0:KC // 2])
                        tb = xpool.tile([P, KC // 2, N], fp8, tag="xt",
                                        name=f"xs{b % 2}b")
                        eng.dma_start(tb[:], xt[b][:, KC // 2:KC])
                        if dr:
                            xacc.append((lambda a, c: lambda k2, ns: (
                                a[:, 2 * k2:2 * k2 + 2, ns] if k2 < KC // 4
                                else c[:, 2 * k2 - KC // 2:2 * k2 - KC // 2 + 2, ns]
                            ))(ta, tb))
                        else:
                            xacc.append((lambda a, c: lambda k, ns: (
                                a[:, k, ns] if k < KC // 2
                                else c[:, k - KC // 2, ns]
                            ))(ta, tb))
                elif kn["xmode"] == "ksplit":
                    # every batch: k-chunks 0..3 on sync, 4..7 on scalar
                    for b in range(B_LOC):
                        ta = xph0.tile([P, KC // 2, N], fp8, tag="xt",
                                       name=f"xka{b}")
                        nc.sync.dma_start(ta[:], xt[b][:, 0:KC // 2])
                        tb = xph1.tile([P, KC // 2, N], fp8, tag="xt",
                                       name=f"xkb{b}")
                        nc.scalar.dma_start(tb[:], xt[b][:, KC // 2:KC])
                        if dr:
                            xacc.append((lambda a, c: lambda k2, ns: (
                                a[:, 2 * k2:2 * k2 + 2, ns] if k2 < KC // 4
                                else c[:, 2 * k2 - KC // 2:2 * k2 - KC // 2 + 2, ns]
                            ))(ta, tb))
                        else:
                            xacc.append((lambda a, c: lambda k, ns: (
                                a[:, k, ns] if k < KC // 2
                                else c[:, k - KC // 2, ns]
                            ))(ta, tb))
                elif kn["q3"]:
                    t0 = xp0.tile([P, KC, N], fp8, tag="xt", name="xb0")
                    nc.sync.dma_start(t0[:], xt[0])
                    t1 = xp1.tile([P, KC, N], fp8, tag="xt", name="xb1")
                    nc.scalar.dma_start(t1[:], xt[1])
                    t2a = xph0.tile([P, KC // 2, N], fp8, tag="xt", name="xb2a")
                    nc.sync.dma_start(t2a[:], xt[2][:, 0:KC // 2])
                    t2b = xph1.tile([P, KC // 2, N], fp8, tag="xt", name="xb2b")
                    nc.scalar.dma_start(t2b[:], xt[2][:, KC // 2:KC])
                    t3 = xp2.tile([P, KC, N], fp8, tag="xt", name="xb3")
                    nc.gpsimd.dma_start(t3[:], xt[3])

                    def acc_whole(t):
                        return lambda k, ns: t[:, k, ns]

                    def acc_whole2(t):
                        return lambda k2, ns: t[:, 2 * k2:2 * k2 + 2, ns]

                    def acc_split2(ta, tb):
                        return lambda k2, ns: (
                            ta[:, 2 * k2:2 * k2 + 2, ns] if k2 < KC // 4
                            else tb[:, 2 * k2 - KC // 2:2 * k2 - KC // 2 + 2, ns]
                        )

                    def acc_split1(ta, tb):
                        return lambda k, ns: (
                            ta[:, k, ns] if k < KC // 2
                            else tb[:, k - KC // 2, ns]
                        )

                    if dr:
                        xacc = [acc_whole2(t0), acc_whole2(t1),
                                acc_split2(t2a, t2b), acc_whole2(t3)]
                    else:
                        xacc = [acc_whole(t0), acc_whole(t1),
                                acc_split1(t2a, t2b), acc_whole(t3)]
                else:
                    for b in range(B_LOC):
                        xpool = (xp0, xp1)[b % 2]
                        xtile = xpool.tile([P, KC, N], fp8, tag="xt",
                                           name=f"xt{b % 2}")
                        eng = nc.scalar if b % 2 else nc.sync
                        eng.dma_start(xtile[:], xt[b])
                        if dr:
                            xacc.append(
                                (lambda t: lambda k2, ns:
                                 t[:, 2 * k2:2 * k2 + 2, ns])(xtile))
                        else:
                            xacc.append(
                                (lambda t: lambda k, ns: t[:, k, ns])(xtile))

                for b in range(B_LOC):
                    rhs = xacc[b]
                    # mains: k-outer so each weight-load serves NCH chunks
                    pss = [
                        pp.tile([P, CHW], f32, tag="ps", name=f"ps{i}")
                        for i in range(NCH)
                    ]
                    if dr:
                        for k2 in range(KC // 2):
                            for ci in range(NCH):
                                nc.tensor.matmul(
                                    pss[ci][:],
                                    ytile[:, b, k2, :, :],
                                    rhs(k2, slice(ci * CHW, (ci + 1) * CHW)),
                                    start=(k2 == 0),
                                    stop=(k2 == KC // 2 - 1),
                                    perf_mode=mybir.MatmulPerfMode.DoubleRow,
                                )
                    else:
                        for k in range(KC):
                            for ci in range(NCH):
                                nc.tensor.matmul(
                                    pss[ci][:],
                                    ytile[:, b, k, :],
                                    rhs(k, slice(ci * CHW, (ci + 1) * CHW)),
                                    start=(k == 0),
                                    stop=(k == KC - 1),
                                )
                    # epilogue per chunk
                    for ci in range(NCH):
                        d2 = d2p.tile([P, CHW], bf16, tag="d2")
                        nc.scalar.activation(
                            d2[:], pss[ci][:],
                            mybir.ActivationFunctionType.Identity,
                            bias=auxs[:, T + b:T + b + 1], scale=1.0,
                        )
                        pst = pt.tile([P, CHW // P, P], bf16, tag="pt")
                        for j in range(CHW // P):
                            nc.tensor.transpose(
                                pst[:, j, :],
                                d2[:, j * P:(j + 1) * P],
                                ident[:],
                            )
                        col = b * NT + ci * (CHW // P)
                        nc.vector.tensor_reduce(
                            mins[:, col:col + CHW // P], pst[:],
                            axis=mybir.AxisListType.X,
                            op=mybir.AluOpType.min,
                        )

                # post: d2min = mins + (x2+2048); cost = sqrt(max(d2min, eps));
                # one Newton step; row-sum -> [128, 1]
                m2 = cons.tile([P, T], f32, tag="m2")
                nc.vector.tensor_add(m2[:], mins[:], x2s)
                nc.vector.tensor_scalar_max(m2[:], m2[:], 1e-20)
                s = cons.tile([P, T], f32, tag="s")
                nc.scalar.sqrt(s[:], m2[:])
                r = cons.tile([P, T], f32, tag="r")
                nc.vector.reciprocal(r[:], s[:])
                t2 = cons.tile([P, T], f32, tag="t2")
                nc.vector.tensor_mul(t2[:], m2[:], r[:])
                nc.vector.tensor_add(t2[:], t2[:], s[:])
                ov = cons.tile([P, 1], f32, tag="ov")
                nc.vector.reduce_sum(ov[:], t2[:], axis=mybir.AxisListType.X)
                nc.vector.tensor_scalar_mul(ov[:], ov[:], 0.5)
                if last or not kn["out_last"]:
                    nc.sync.dma_start(out[:], ov[:])

            for ri in range(reps):
                emit_body(ri == reps - 1)

    nc.compile()
    return nc


def _get_nc(reps: int = 1, **knobs):
    key = ("nc", reps, tuple(sorted(knobs.items())))
    if key not in _CACHE:
        _CACHE[key] = _build_nc(reps, **knobs)
    return _CACHE[key]


def make_in_maps(image_features: np.ndarray, token_ids: np.ndarray,
                 emb_table: np.ndarray, **knobs) -> list[dict]:
    """Shard + lay out the full inputs into per-core device input maps."""
    x = np.asarray(image_features, dtype=np.float32)
    tok = np.asarray(token_ids)
    emb = np.asarray(emb_table, dtype=np.float32)

    in_maps = []
    for c in range(N_CORES):
        xc = x[c * B_LOC:(c + 1) * B_LOC]                       # [4, N, D]
        # x^T: [b, p, k, n] = x[b, n, k*128 + p]
        xT = np.ascontiguousarray(xc.transpose(0, 2, 1))        # [4, D, N]
        xT = xT.reshape(B_LOC, KC, P, N)
        xt_dev = np.ascontiguousarray(
            xT.transpose(0, 2, 1, 3)).astype(FP8)               # [4, P, KC, N]
        xt1_dev = np.ascontiguousarray(
            xT.transpose(2, 0, 1, 3)).astype(FP8)               # [P, 4, KC, N]
        # exact fp32 row norms (+2048), laid out [p, b*NT+t]
        x2 = np.square(xc).sum(axis=-1, dtype=np.float64) + 2048.0  # [4, N]
        x2t_dev = (
            x2.reshape(B_LOC, NT, P).transpose(2, 0, 1).reshape(P, T)
        ).astype(np.float32)

        y = emb[tok[c * B_LOC:(c + 1) * B_LOC]]                 # [4, L, D]
        yT = (-2.0 * y).transpose(0, 2, 1)                      # [4, D, L]
        yT = yT.reshape(B_LOC, KC, P, L)
        # DoubleRow fp8: [p, b, k2, ko, l]
        ytdr_dev = np.ascontiguousarray(
            yT.reshape(B_LOC, KC // 2, 2, P, L).transpose(3, 0, 1, 2, 4)
        ).astype(FP8)
        # bf16: [p, b, k, l]
        ytb_dev = np.ascontiguousarray(
            yT.transpose(2, 0, 1, 3)).astype(BF16)
        y2 = np.square(y).sum(axis=-1, dtype=np.float64) - 2048.0  # [4, L]
        y2t_dev = (y2.transpose(1, 0)).astype(np.float32)       # [L, 4]
        aux_dev = np.ascontiguousarray(
            np.concatenate([x2t_dev, y2t_dev], axis=1))         # [P, T + B_LOC]

        in_maps.append({
            "xt": xt_dev,
            "xt1": xt1_dev,
            "ytdr": ytdr_dev,
            "ytb": ytb_dev,
            "aux": aux_dev,
        })
    return in_maps


def kernel(image_features: np.ndarray, token_ids: np.ndarray,
           emb_table: np.ndarray) -> np.ndarray:
    from concourse import mybir
    from concourse.bass_utils import run_bass_kernel_spmd

    nc = _get_nc()
    declared = {
        alloc.memorylocations[0].name
        for alloc in nc.m.functions[0].allocations
        if isinstance(alloc, mybir.MemoryLocationSet)
        and alloc.kind == "ExternalInput"
    }
    in_maps = [
        {k: v for k, v in m.items() if k in declared}
        for m in make_in_maps(image_features, token_ids, emb_table)
    ]
    res = run_bass_kernel_spmd(nc, in_maps, core_ids=list(range(N_CORES)))
    total = np.float64(0.0)
    for c in range(N_CORES):
        total += res.results[c]["out"].astype(np.float64).sum()
    return np.float32(total / (B * N))


# revision 37
# speedup vs baseline: 1.1003x; 1.0593x over previous
"""Trainium2 Bass kernel for nn_Captioner_41412074668572 (retrieval_knn).

Computes: mean over (b, n) of min over l of ||image_features[b,n] - emb_table[token_ids[b,l]]||_2

Strategy (8 NeuronCores, data-parallel over batch B=32 -> 4 batches/core):
  y-stationary matmul: weights = (-2*y)^T tiles [d=128, l=128] (reused across
  2048 streamed x columns -> LDWEIGHTS amortized 16x vs the x-stationary
  form), rhs = x^T fp8 [d=128, n=512] slices, PSUM accumulates -2*x.y over
  the d=1024 contraction into [l=128, n=512] chunks.  With dr=True both
  operands are fp8 and pairs of k-chunks run as one DoubleRow matmul
  (2 MACs/cell/cycle).
  Epilogue: ACT copies PSUM->SBUF in bf16 adding the per-partition bias
  y2[l] - 2048 (centering d2' near 0 keeps bf16 quantization harmless);
  PE transposes each [128,128] block (data as stationary, identity moving);
  DVE min-reduces the transposed [n=128, 4, l=128] banks over l; then
  m2 = mins + (x2+2048), clamp, sqrt (+1 Newton step), row-sum -> [128,1].
  Host sums 8*[128] partials in float64 and divides by B*N.
"""

import numpy as np
import ml_dtypes

B, N, L, D, V = 32, 2048, 128, 1024, 32000
N_CORES = 8
B_LOC = B // N_CORES          # 4 batches per core
P = 128                       # partitions
KC = D // P                   # 8 contraction chunks of 128
NT = N // P                   # 16 n-tiles of 128 per batch
T = B_LOC * NT                # 64 mins columns per core
NCH = 4                       # n-chunks of 512 per batch
CHW = N // NCH                # chunk width 512

_CACHE: dict = {}

BF16 = ml_dtypes.bfloat16
FP8 = ml_dtypes.float8_e4m3


DEFAULT_KNOBS = dict(
    dr=True,          # fp8 DoubleRow mains (both operands fp8)
    xmode="two",      # "two": 2x4.2MB batch-pair DMAs on sync; "one": 8.4MB;
                      # "sync1": per-batch on sync; "whole": dual-queue
    q3=False,         # spread x over all 3 DMA queues (else 2-queue alternate)
    bufs_x0=5,        # x pair slots, sync-queue pool (2.5 reps of x buffered)
    bufs_x1=4,        # whole-batch x slots, scalar-queue pool (non-sync1)
    bufs_x2=2,        # whole-batch x slots, gpsimd-queue pool (q3 only)
    bufs_xh=2,        # half-batch x slots (ksplit / q3-b2 pools)
    bufs_d2=3,
    bufs_ps=4,        # main psum chunk tiles
    bufs_pt=3,        # transpose psum tiles
    y_eng="sync",     # engine for the y+aux DMAs ("sync": single-queue —
                      # zero SDMA queue-switching; "gpsimd": separate SWDGE)
    out_last=True,    # emit the (512B) out DMA only on the last rep
)


def _build_nc(reps: int = 1, **knobs):
    """Build the Bass program. `reps` unrolls the whole body N times inside
    one NEFF (used only for marginal-time measurement in test.py)."""
    import concourse.tile as tile
    from concourse import bacc, mybir
    from concourse.masks import make_identity

    kn = dict(DEFAULT_KNOBS)
    kn.update(knobs)

    f32 = mybir.dt.float32
    bf16 = mybir.dt.bfloat16
    fp8 = mybir.dt.float8e4

    dr = kn["dr"]

    nc = bacc.Bacc("TRN2", target_bir_lowering=False, debug=False,
                   num_devices=N_CORES)

    # x^T fp8: [b, p, k, n] with x[b, n, k*128+p]
    if kn["xmode"] in ("one", "two"):
        # partition-outermost over ALL batches: one/two giant contiguous
        # DMAs (per-transfer fixed overhead ~0.67us caps 2MB transfers at
        # ~370 GB/s; 4-8MB transfers reach ~400-425)
        xt1 = nc.dram_tensor("xt1", [P, B_LOC, KC, N], fp8,
                             kind="ExternalInput")
    else:
        xt = nc.dram_tensor("xt", [B_LOC, P, KC, N], fp8,
                            kind="ExternalInput")
    if dr:
        # y packed for DoubleRow: [p, b, k2, ko, l] = -2*y[b, l, (2*k2+ko)*128+p]
        yt = nc.dram_tensor("ytdr", [P, B_LOC, KC // 2, 2, L], fp8,
                            kind="ExternalInput")
    else:
        # y bf16: [p, b, k, l] = -2*y[b, l, k*128+p]
        yt = nc.dram_tensor("ytb", [P, B_LOC, KC, L], bf16,
                            kind="ExternalInput")
    # aux: cols [0, T) = x2 + 2048 arranged [p, b*NT + t] matching mins
    # columns; cols [T, T + B_LOC) = y2[l] - 2048 per batch (partition = l).
    aux = nc.dram_tensor("aux", [P, T + B_LOC], f32, kind="ExternalInput")
    out = nc.dram_tensor("out", [P, 1], f32, kind="ExternalOutput")

    with tile.TileContext(nc) as tc:
        with (
            tc.tile_pool(name="xp0", bufs=kn["bufs_x0"]) as xp0,
            tc.tile_pool(name="xp1", bufs=kn["bufs_x1"]) as xp1,
            tc.tile_pool(name="xp2", bufs=kn["bufs_x2"]) as xp2,
            tc.tile_pool(name="xph0", bufs=kn["bufs_xh"]) as xph0,
            tc.tile_pool(name="xph1", bufs=kn["bufs_xh"]) as xph1,
            tc.tile_pool(name="yp", bufs=2) as yp,
            tc.tile_pool(name="cons", bufs=2) as cons,
            tc.tile_pool(name="idn", bufs=1) as idn,
            tc.tile_pool(name="d2p", bufs=kn["bufs_d2"]) as d2p,
            tc.tile_pool(name="ps", bufs=kn["bufs_ps"], space="PSUM") as pp,
            tc.tile_pool(name="pt", bufs=kn["bufs_pt"], space="PSUM") as pt,
        ):
            ident = idn.tile([P, P], bf16, tag="ident")
            make_identity(nc, ident[:])

            def emit_body(last: bool):
                # small constants first
                auxs = cons.tile([P, T + B_LOC], f32, tag="auxs")
                getattr(nc, kn["y_eng"]).dma_start(auxs[:], aux[:])
                x2s = auxs[:, 0:T]
                y2s = auxs[:, T:T + B_LOC]
                if dr:
                    ytile = yp.tile([P, B_LOC, KC // 2, 2, L], fp8, tag="yt")
                else:
                    ytile = yp.tile([P, B_LOC, KC, L], bf16, tag="yt")
                getattr(nc, kn["y_eng"]).dma_start(ytile[:], yt[:])

                mins = cons.tile([P, T], f32, tag="mins")

                # x DMA plan: per batch a k-slice accessor rhs(k, nslice).
                # q3 spreads the 8.4 MB x stream over all three DMA queues
                # (sync / scalar / gpsimd-SWDGE): b0 sync, b1 scalar,
                # b2 k-split across sync+scalar, b3 gpsimd.
                xacc = []
                if kn["xmode"] == "one":
                    # one 8.4MB x DMA per rep (max transfer efficiency);
                    # bufs_x0=2 -> next rep's DMA streams during this rep
                    ta = xp0.tile([P, B_LOC, KC, N], fp8, tag="xt",
                                  name="xall")
                    nc.sync.dma_start(ta[:], xt1[:])
                    for b in range(B_LOC):
                        if dr:
                            xacc.append((lambda t, bb: lambda k2, ns:
                                         t[:, bb, 2 * k2:2 * k2 + 2, ns])(ta, b))
                        else:
                            xacc.append((lambda t, bb: lambda k, ns:
                                         t[:, bb, k, ns])(ta, b))
                elif kn["xmode"] == "two":
                    # two 4.2MB x DMAs per rep (batch pairs)
                    for g in range(2):
                        tg = xp0.tile([P, 2, KC, N], fp8, tag="xt",
                                      name="xpair")
                        nc.sync.dma_start(tg[:], xt1[:, 2 * g:2 * g + 2])
                        for bb in range(2):
                            if dr:
                                xacc.append((lambda t, b2: lambda k2, ns:
                                             t[:, b2, 2 * k2:2 * k2 + 2, ns])(tg, bb))
                            else:
                                xacc.append((lambda t, b2: lambda k, ns:
                                             t[:, b2, k, ns])(tg, bb))
                elif kn["xmode"] == "sync1":
                    # all x on the sync HWDGE ring: a single queue saturates
                    # all 16 SDMA engines, and the scalar engine then carries
                    # ONLY the ACTIVATEs -- a scalar-issued DMA that waits on
                    # a buffer-free semaphore would head-of-line-block the
                    # PSUM-draining ACTs behind it (strict 8-deep FIFO).
                    for b in range(B_LOC):
                        xtile = xp0.tile([P, KC, N], fp8, tag="xt", name="xs")
                        nc.sync.dma_start(xtile[:], xt[b])
                        if dr:
                            xacc.append((lambda t: lambda k2, ns:
                                         t[:, 2 * k2:2 * k2 + 2, ns])(xtile))
                        else:
                            xacc.append((lambda t: lambda k, ns:
                                         t[:, k, ns])(xtile))
                elif kn["xmode"] == "ksplit2":
                    # per batch: two 1MB k-half DMAs on the batch's queue, so
                    # the k2<2 matmuls can start when the first half lands
                    for b in range(B_LOC):
                        xpool = (xph0, xph1)[b % 2]
                        eng = (nc.sync, nc.scalar)[b % 2]
                        ta = xpool.tile([P, KC // 2, N], fp8, tag="xt",
                                        name=f"xs{b % 2}a")
                        eng.dma_start(ta[:], xt[b][:, 0:KC // 2])
                        tb = xpool.tile([P, KC // 2, N], fp8, tag="xt",
                                        name=f"xs{b % 2}b")
                        eng.dma_start(tb[:], xt[b][:, KC // 2:KC])
                        if dr:
                            xacc.append((lambda a, c: lambda k2, ns: (
                                a[:, 2 * k2:2 * k2 + 2, ns] if k2 < KC // 4
                                else c[:, 2 * k2 - KC // 2:2 * k2 - KC // 2 + 2, ns]
                            ))(ta, tb))
                        else:
                            xacc.append((lambda a, c: lambda k, ns: (
                                a[:, k, ns] if k < KC // 2
                                else c[:, k - KC // 2, ns]
                            ))(ta, tb))
                elif kn["xmode"] == "ksplit":
                    # every batch: k-chunks 0..3 on sync, 4..7 on scalar
                    for b in range(B_LOC):
                        ta = xph0.tile([P, KC // 2, N], fp8, tag="xt",
                                       name=f"xka{b}")
                        nc.sync.dma_start(ta[:], xt[b][:, 0:KC // 2])
                        tb = xph1.tile([P, KC // 2, N], fp8, tag="xt",
                                       name=f"xkb{b}")
                        nc.scalar.dma_start(tb[:], xt[b][:, KC // 2:KC])
                        if dr:
                            xacc.append((lambda a, c: lambda k2, ns: (
                                a[:, 2 * k2:2 * k2 + 2, ns] if k2 < KC // 4
                                else c[:, 2 * k2 - KC // 2:2 * k2 - KC // 2 + 2, ns]
                            ))(ta, tb))
                        else:
                            xacc.append((lambda a, c: lambda k, ns: (
                                a[:, k, ns] if k < KC // 2
                                else c[:, k - KC // 2, ns]
                            ))(ta, tb))
                elif kn["q3"]:
                    t0 = xp0.tile([P, KC, N], fp8, tag="xt", name="xb0")
                    nc.sync.dma_start(t0[:], xt[0])
                    t1 = xp1.tile([P, KC, N], fp8, tag="xt", name="xb1")
                    nc.scalar.dma_start(t1[:], xt[1])
                    t2a = xph0.tile([P, KC // 2, N], fp8, tag="xt", name="xb2a")
                    nc.sync.dma_start(t2a[:], xt[2][:, 0:KC // 2])
                    t2b = xph1.tile([P, KC // 2, N], fp8, tag="xt", name="xb2b")
                    nc.scalar.dma_start(t2b[:], xt[2][:, KC // 2:KC])
                    t3 = xp2.tile([P, KC, N], fp8, tag="xt", name="xb3")
                    nc.gpsimd.dma_start(t3[:], xt[3])

                    def acc_whole(t):
                        return lambda k, ns: t[:, k, ns]

                    def acc_whole2(t):
                        return lambda k2, ns: t[:, 2 * k2:2 * k2 + 2, ns]

                    def acc_split2(ta, tb):
                        return lambda k2, ns: (
                            ta[:, 2 * k2:2 * k2 + 2, ns] if k2 < KC // 4
                            else tb[:, 2 * k2 - KC // 2:2 * k2 - KC // 2 + 2, ns]
                        )

                    def acc_split1(ta, tb):
                        return lambda k, ns: (
                            ta[:, k, ns] if k < KC // 2
                            else tb[:, k - KC // 2, ns]
                        )

                    if dr:
                        xacc = [acc_whole2(t0), acc_whole2(t1),
                                acc_split2(t2a, t2b), acc_whole2(t3)]
                    else:
                        xacc = [acc_whole(t0), acc_whole(t1),
                                acc_split1(t2a, t2b), acc_whole(t3)]
                else:
                    for b in range(B_LOC):
                        xpool = (xp0, xp1)[b % 2]
                        xtile = xpool.tile([P, KC, N], fp8, tag="xt",
                                           name=f"xt{b % 2}")
                        eng = nc.scalar if b % 2 else nc.sync
                        eng.dma_start(xtile[:], xt[b])
                        if dr:
                            xacc.append(
                                (lambda t: lambda k2, ns:
                                 t[:, 2 * k2:2 * k2 + 2, ns])(xtile))
                        else:
                            xacc.append(
                                (lambda t: lambda k, ns: t[:, k, ns])(xtile))

                for b in range(B_LOC):
                    rhs = xacc[b]
                    # mains: k-outer so each weight-load serves NCH chunks
                    pss = [
                        pp.tile([P, CHW], f32, tag="ps", name=f"ps{i}")
                        for i in range(NCH)
                    ]
                    if dr:
                        for k2 in range(KC // 2):
                            for ci in range(NCH):
                                nc.tensor.matmul(
                                    pss[ci][:],
                                    ytile[:, b, k2, :, :],
                                    rhs(k2, slice(ci * CHW, (ci + 1) * CHW)),
                                    start=(k2 == 0),
                                    stop=(k2 == KC // 2 - 1),
                                    perf_mode=mybir.MatmulPerfMode.DoubleRow,
                                )
                    else:
                        for k in range(KC):
                            for ci in range(NCH):
                                nc.tensor.matmul(
                                    pss[ci][:],
                                    ytile[:, b, k, :],
                                    rhs(k, slice(ci * CHW, (ci + 1) * CHW)),
                                    start=(k == 0),
                                    stop=(k == KC - 1),
                                )
                    # epilogue per chunk
                    for ci in range(NCH):
                        d2 = d2p.tile([P, CHW], bf16, tag="d2")
                        nc.scalar.activation(
                            d2[:], pss[ci][:],
                            mybir.ActivationFunctionType.Identity,
                            bias=auxs[:, T + b:T + b + 1], scale=1.0,
                        )
                        pst = pt.tile([P, CHW // P, P], bf16, tag="pt")
                        for j in range(CHW // P):
                            nc.tensor.transpose(
                                pst[:, j, :],
                                d2[:, j * P:(j + 1) * P],
                                ident[:],
                            )
                        col = b * NT + ci * (CHW // P)
                        nc.vector.tensor_reduce(
                            mins[:, col:col + CHW // P], pst[:],
                            axis=mybir.AxisListType.X,
                            op=mybir.AluOpType.min,
                        )

                # post: d2min = mins + (x2+2048); cost = sqrt(max(d2min, eps));
                # one Newton step; row-sum -> [128, 1]
                m2 = cons.tile([P, T], f32, tag="m2")
                nc.vector.tensor_add(m2[:], mins[:], x2s)
                nc.vector.tensor_scalar_max(m2[:], m2[:], 1e-20)
                s = cons.tile([P, T], f32, tag="s")
                nc.scalar.sqrt(s[:], m2[:])
                r = cons.tile([P, T], f32, tag="r")
                nc.vector.reciprocal(r[:], s[:])
                t2 = cons.tile([P, T], f32, tag="t2")
                nc.vector.tensor_mul(t2[:], m2[:], r[:])
                nc.vector.tensor_add(t2[:], t2[:], s[:])
                ov = cons.tile([P, 1], f32, tag="ov")
                nc.vector.reduce_sum(ov[:], t2[:], axis=mybir.AxisListType.X)
                nc.vector.tensor_scalar_mul(ov[:], ov[:], 0.5)
                if last or not kn["out_last"]:
                    nc.sync.dma_start(out[:], ov[:])

            for ri in range(reps):
                emit_body(ri == reps - 1)

    nc.compile()
    return nc


def _get_nc(reps: int = 1, **knobs):
    key = ("nc", reps, tuple(sorted(knobs.items())))
    if key not in _CACHE:
        _CACHE[key] = _build_nc(reps, **knobs)
    return _CACHE[key]


def make_in_maps(image_features: np.ndarray, token_ids: np.ndarray,
                 emb_table: np.ndarray, **knobs) -> list[dict]:
    """Shard + lay out the full inputs into per-core device input maps."""
    x = np.asarray(image_features, dtype=np.float32)
    tok = np.asarray(token_ids)
    emb = np.asarray(emb_table, dtype=np.float32)

    in_maps = []
    for c in range(N_CORES):
        xc = x[c * B_LOC:(c + 1) * B_LOC]                       # [4, N, D]
        # x^T: [b, p, k, n] = x[b, n, k*128 + p]
        xT = np.ascontiguousarray(xc.transpose(0, 2, 1))        # [4, D, N]
        xT = xT.reshape(B_LOC, KC, P, N)
        xt_dev = np.ascontiguousarray(
            xT.transpose(0, 2, 1, 3)).astype(FP8)               # [4, P, KC, N]
        xt1_dev = np.ascontiguousarray(
            xT.transpose(2, 0, 1, 3)).astype(FP8)               # [P, 4, KC, N]
        # exact fp32 row norms (+2048), laid out [p, b*NT+t]
        x2 = np.square(xc).sum(axis=-1, dtype=np.float64) + 2048.0  # [4, N]
        x2t_dev = (
            x2.reshape(B_LOC, NT, P).transpose(2, 0, 1).reshape(P, T)
        ).astype(np.float32)

        y = emb[tok[c * B_LOC:(c + 1) * B_LOC]]                 # [4, L, D]
        yT = (-2.0 * y).transpose(0, 2, 1)                      # [4, D, L]
        yT = yT.reshape(B_LOC, KC, P, L)
        # DoubleRow fp8: [p, b, k2, ko, l]
        ytdr_dev = np.ascontiguousarray(
            yT.reshape(B_LOC, KC // 2, 2, P, L).transpose(3, 0, 1, 2, 4)
        ).astype(FP8)
        # bf16: [p, b, k, l]
        ytb_dev = np.ascontiguousarray(
            yT.transpose(2, 0, 1, 3)).astype(BF16)
        y2 = np.square(y).sum(axis=-1, dtype=np.float64) - 2048.0  # [4, L]
        y2t_dev = (y2.transpose(1, 0)).astype(np.float32)       # [L, 4]
        aux_dev = np.ascontiguousarray(
            np.concatenate([x2t_dev, y2t_dev], axis=1))         # [P, T + B_LOC]

        in_maps.append({
            "xt": xt_dev,
            "xt1": xt1_dev,
            "ytdr": ytdr_dev,
            "ytb": ytb_dev,
            "aux": aux_dev,
        })
    return in_maps


def kernel(image_features: np.ndarray, token_ids: np.ndarray,
           emb_table: np.ndarray) -> np.ndarray:
    from concourse import mybir
    from concourse.bass_utils import run_bass_kernel_spmd

    nc = _get_nc()
    declared = {
        alloc.memorylocations[0].name
        for alloc in nc.m.functions[0].allocations
        if isinstance(alloc, mybir.MemoryLocationSet)
        and alloc.kind == "ExternalInput"
    }
    in_maps = [
        {k: v for k, v in m.items() if k in declared}
        for m in make_in_maps(image_features, token_ids, emb_table)
    ]
    res = run_bass_kernel_spmd(nc, in_maps, core_ids=list(range(N_CORES)))
    total = np.float64(0.0)
    for c in range(N_CORES):
        total += res.results[c]["out"].astype(np.float64).sum()
    return np.float32(total / (B * N))
